# revision 17
# baseline (speedup 1.0000x reference)
"""DFL (distribution focal loss) Trainium2 Bass kernel.

Problem: pred [N,64] f32 (N=1048576, 4 sides x 16 bins), target [N,4] f32 in
[0,16), weight [N,1] f32.  loss = mean over [N,4] of w * KL(twohot(t) || softmax(p)).

Per-site (site = one (row, side), 16 bins):
    f   = frac(t);  l = floor(t)
    ent = f*log f + (1-f)*log(1-f)          (xlogy semantics)
    lse = log(sum_k exp(p_k))
    dot = sum_k p_k * relu(1 - |t - k|)     (= (1-f) p_l + f p_{l+1})
    kl  = ent + lse - dot
loss = sum(w*kl) / (4N)

Sharding: rows split across 8 cores (trivially data-parallel); scalar partials
are summed on host.

Device layout (per core): sites = rows*4 = 524288, viewed as [128 partitions,
Q=4096 sites], each partition owning contiguous sites. Per tile of S sites per
partition:
  - ACT: e = exp(p) (bf16 out), lse = log(s), entropy logs
  - DVE: one fused custom op  y = p * relu(min(u2, 2-u2)), u2 = t' - Idx,
    where t' = t + 1 + 16*(site % CHUNK) streams via a stride-0 broadcast AP
  - PE : segmented 16->1 sums of e and y as 16 PSUM-accumulating identity
    matmuls over strided views (out[m,n] = rhs[m,n] with lhsT = I)
  - DVE tensor_tensor_reduce with chained per-partition accumulators folds
    w * (ent + lse) and w * dot into [128,1] partials; host sums them.
"""

import numpy as np

REG_MAX = 16
N_ROWS = 1_048_576
NCORES = 8
P = 128
ROWS_PER_CORE = N_ROWS // NCORES          # 131072
Q_FULL = ROWS_PER_CORE * 4 // P           # 4096 sites per partition
S_FULL = 512                              # sites per partition per tile
CHUNK_FULL = 128                          # sites per custom-op instruction
EPS = 1e-12

_DFL_OP = None
_FRAC_OP = None
_BUILT = None


def _register_op(name, spec):
    """Register a custom DVE op at runtime, auto-pinning its uops sha."""
    import concourse.dve_ops as dve_ops

    for o in dve_ops.OPS:
        if o.name == name:
            return o
    dve_ops._SUB_OPCODE_FOR_NAME[name] = (
        max(dve_ops._SUB_OPCODE_FOR_NAME.values()) + 1
    )
    assert dve_ops._SUB_OPCODE_FOR_NAME[name] < 0x20
    shas = {}
    for ver in ("v3", "v4"):
        probe = dve_ops.DveOp(name, spec, subdim=False, uops_sha={})
        try:
            probe.compile(ver)
        except ValueError as e:
            import re

            m = re.search(r"\(%s: ([0-9a-f]+)" % ver, str(e))
            if m is None:
                raise
            shas[ver] = m.group(1)
        dve_ops._COMPILE_CACHE.pop((name, ver), None)
    op = dve_ops.DveOp(name, spec, subdim=False, uops_sha=shas)
    dve_ops.OPS.append(op)
    dve_ops.CUSTOM_DVE_SPECS[name] = spec
    return op


def _get_dfl_op():
    """Register the fused two-hot dot op: out = Src0 * relu(min(u2, 2-u2)),
    u2 = Src1 - Idx. With Src1 = 1 + t + 16*(site index), this yields
    p * relu(1 - |t - k|) for bin k."""
    global _DFL_OP
    if _DFL_OP is not None:
        return _DFL_OP
    from concourse.dve_spec import Spec, Src0, Src1, Zero, One, Idx, maxx, minn

    u2 = Src1 - Idx
    two = One + One
    hat = maxx(minn(u2, two - u2), Zero)

    def _ref(in0, in1, c0, c1, c2):
        x = np.asarray(in0, np.float32)
        pp = x.shape[0]
        xf = x.reshape(pp, -1)
        tb = np.asarray(in1, np.float32).reshape(pp, -1)
        idx = np.arange(xf.shape[1], dtype=np.float32)[None, :]
        u2v = tb - idx
        hatv = np.maximum(np.minimum(u2v, 2.0 - u2v), 0.0)
        return (xf.astype(np.float32) * hatv).reshape(x.shape)

    _DFL_OP = _register_op("DFL_TWOHOT_ANT", Spec(body=Src0 * hat, reference=_ref))
    return _DFL_OP


def _get_frac_op():
    """frac(x) for x in [0, 2^22): r = (x + 2^23) - 2^23 rounds to nearest
    integer; out = (x - r) + (r > x) = x - floor(x)."""
    global _FRAC_OP
    if _FRAC_OP is not None:
        return _FRAC_OP
    from concourse.dve_spec import Spec, Src0, C0

    r = (Src0 + C0) - C0
    body = (Src0 - r) + (r > Src0)

    def _ref(in0, in1, c0, c1, c2):
        x = np.asarray(in0, np.float32)
        c = np.float32(c0 if not isinstance(c0, np.ndarray) else c0.flat[0])
        rr = ((x + c).astype(np.float32) - c).astype(np.float32)
        return ((x - rr) + (rr > x)).astype(np.float32)

    _FRAC_OP = _register_op("FRAC_FLOOR_ANT", Spec(body=body, reference=_ref))
    return _FRAC_OP


def build_nc(Q=Q_FULL, S=S_FULL, CHUNK=CHUNK_FULL):
    """Build the single-core Bass program (same NEFF on all cores)."""
    import concourse.bass as bass
    import concourse.bacc as bacc
    import concourse.tile as tile
    from concourse import mybir

    f32 = mybir.dt.float32
    bf16 = mybir.dt.bfloat16
    AF = mybir.ActivationFunctionType
    ALU = mybir.AluOpType
    T = Q // S
    R = S // 4
    F16 = S * 16
    assert Q % S == 0 and S % CHUNK == 0 and S % 4 == 0 and CHUNK % 4 == 0

    from concourse.dve_ops import TENSOR_TENSOR_REDUCE as ttr_op

    # Steer ACT table-set choice to the one set containing BOTH exp and ln,
    # so Exp/Ln interleaving doesn't thrash ACT_TABLE_LOAD (~2.7us each).
    # Indices (= act_func_set_id) are preserved; single-function sets are
    # emptied so the chooser can't pick them.
    import concourse.hw_specs as hw_specs

    if not getattr(bacc, "_dfl_act_tables_patched", False):
        _orig_gat = hw_specs.get_activation_tables

        def _gat(arch):
            tabs = _orig_gat(arch)
            AF_ = mybir.ActivationFunctionType
            combined = {
                n for n, fns in tabs.items() if AF_.Exp in fns and AF_.Ln in fns
            }
            if combined:
                out = {}
                for n, fns in tabs.items():
                    if (AF_.Exp in fns or AF_.Ln in fns) and n not in combined:
                        fns = set()
                    out[n] = fns
                return out
            return tabs

        bacc.get_activation_tables = _gat
        bacc._dfl_act_tables_patched = True

    op = _get_dfl_op()
    frac_op = _get_frac_op()
    nc = bacc.Bacc("TRN2")
    pred_d = nc.dram_tensor("pred", [P, Q * 16], f32, kind="ExternalInput")
    t_d = nc.dram_tensor("tgt", [P, Q], f32, kind="ExternalInput")
    w_d = nc.dram_tensor("wgt", [P, Q // 4], f32, kind="ExternalInput")
    id_d = nc.dram_tensor("ident", [P, P], bf16, kind="ExternalInput")
    kr_d = nc.dram_tensor("kramp", [P, S], f32, kind="ExternalInput")
    out_d = nc.dram_tensor("out", [P, 2], f32, kind="ExternalOutput")

    with tile.TileContext(nc) as tc:
        with (
            tc.tile_pool(name="big", bufs=2) as big,
            tc.tile_pool(name="med", bufs=2) as med,
            tc.tile_pool(name="small", bufs=2) as small,
            tc.tile_pool(name="consts", bufs=1) as consts,
            tc.tile_pool(name="accp", bufs=1) as accp,
            tc.tile_pool(name="psum", bufs=2, space="PSUM") as psum,
        ):
            ident = consts.tile([P, P], bf16, tag="ident")
            nc.gpsimd.dma_start(ident[:], id_d[:])
            kramp = consts.tile([P, S], f32, tag="kramp")
            nc.gpsimd.dma_start(kramp[:], kr_d[:])
            eps_b = consts.tile([P, 1], f32, tag="eps_b")
            nc.gpsimd.memset(eps_b[:], EPS)
            one_b = consts.tile([P, 1], f32, tag="one_b")
            nc.gpsimd.memset(one_b[:], 1.0 + EPS)
            accA = accp.tile([P, T], f32, tag="accA")
            accB = accp.tile([P, T], f32, tag="accB")
            junk = accp.tile([P, S], f32, tag="junk")

            for t in range(T):
                pt = big.tile([P, F16], f32, tag="pred")
                nc.gpsimd.dma_start(pt[:], pred_d[:, t * F16 : (t + 1) * F16])
                tt = small.tile([P, S], f32, tag="t")
                nc.gpsimd.dma_start(tt[:], t_d[:, t * S : (t + 1) * S])
                wt = small.tile([P, R], f32, tag="w")
                nc.gpsimd.dma_start(wt[:], w_d[:, t * R : (t + 1) * R])

                # e2/y2 are "bin-major": address(s, k) = k*S + s, so each
                # bin-k slice [P, S] is contiguous for the PE moving fetch.
                e = med.tile([P, F16], bf16, tag="e")
                e_bm = e[:].rearrange("p (k s) -> p s k", s=S)
                nc.scalar.activation(e_bm, pt[:], AF.Exp)

                tp = small.tile([P, S], f32, tag="tp")
                nc.vector.tensor_add(tp[:], tt[:], kramp[:])

                y = med.tile([P, F16], bf16, tag="y")
                y_bm = y[:].rearrange("p (k s) -> p s k", s=S)
                for c in range(S // CHUNK):
                    cs = c * CHUNK
                    in1 = (
                        tp[:, cs : cs + CHUNK]
                        .unsqueeze(2)
                        .broadcast_to([P, CHUNK, 16])
                    )
                    nc.vector._custom_dve(
                        op,
                        out=y_bm[:, cs : cs + CHUNK, :],
                        in0=pt[:, cs * 16 : (cs + CHUNK) * 16],
                        in1=in1,
                    )

                se = psum.tile([P, S], f32, tag="se")
                for k in range(16):
                    nc.tensor.matmul(
                        se[:], ident[:], e[:, k * S : (k + 1) * S],
                        start=(k == 0), stop=(k == 15),
                    )
                yd = psum.tile([P, S], f32, tag="yd")
                for k in range(16):
                    nc.tensor.matmul(
                        yd[:], ident[:], y[:, k * S : (k + 1) * S],
                        start=(k == 0), stop=(k == 15),
                    )

                lse = small.tile([P, S], f32, tag="lse")
                nc.scalar.activation(lse[:], se[:], AF.Ln)

                fr = small.tile([P, S], f32, tag="fr")
                nc.vector._custom_dve(
                    frac_op, out=fr[:], in0=tp[:], s0=float(2**23)
                )
                lf = small.tile([P, S], f32, tag="lf")
                nc.scalar.activation(lf[:], fr[:], AF.Ln, bias=eps_b[:])
                l1f = small.tile([P, S], f32, tag="l1f")
                nc.scalar.activation(l1f[:], fr[:], AF.Ln, bias=one_b[:], scale=-1.0)

                d2 = small.tile([P, S], f32, tag="d2")
                nc.gpsimd.tensor_sub(d2[:], lf[:], l1f[:])
                m1 = small.tile([P, S], f32, tag="m1")
                nc.gpsimd.tensor_mul(m1[:], fr[:], d2[:])
                ent = small.tile([P, S], f32, tag="ent")
                nc.gpsimd.tensor_add(ent[:], m1[:], l1f[:])
                kl = small.tile([P, S], f32, tag="kl")
                nc.gpsimd.tensor_add(kl[:], ent[:], lse[:])

                wb = wt[:].unsqueeze(2).broadcast_to([P, R, 4])
                klv = kl[:].rearrange("p (r j) -> p r j", j=4)
                jv = junk[:].rearrange("p (r j) -> p r j", j=4)
                nc.vector._custom_dve(
                    ttr_op, out=jv, in0=klv, in1=wb, s0=0.0, s1=1.0,
                    accum_out=accA[:, t : t + 1],
                )
                ydv = yd[:].rearrange("p (r j) -> p r j", j=4)
                nc.vector._custom_dve(
                    ttr_op, out=jv, in0=ydv, in1=wb, s0=0.0, s1=1.0,
                    accum_out=accB[:, t : t + 1],
                )

            sumA = accp.tile([P, 1], f32, tag="sumA")
            sumB = accp.tile([P, 1], f32, tag="sumB")
            nc.vector.tensor_reduce(
                sumA[:], accA[:], axis=mybir.AxisListType.X, op=ALU.add
            )
            nc.vector.tensor_reduce(
                sumB[:], accB[:], axis=mybir.AxisListType.X, op=ALU.add
            )
            outt = accp.tile([P, 2], f32, tag="outt")
            nc.vector.tensor_copy(outt[:, 0:1], sumA[:])
            nc.vector.tensor_copy(outt[:, 1:2], sumB[:])
            nc.gpsimd.dma_start(out_d[:], outt[:])

    nc.finalize()
    return nc


def make_host_inputs(pred, target, weight, Q=Q_FULL, S=S_FULL, CHUNK=CHUNK_FULL,
                     ncores=NCORES):
    """Shard full inputs into per-core in_maps."""
    import ml_dtypes

    pred = np.ascontiguousarray(np.asarray(pred, np.float32))
    target = np.ascontiguousarray(np.asarray(target, np.float32))
    weight = np.ascontiguousarray(np.asarray(weight, np.float32))
    rows = pred.shape[0]
    rows_per_core = rows // ncores
    ident = np.eye(P, dtype=ml_dtypes.bfloat16)
    ramp = (1.0 + 16.0 * (np.arange(S) % CHUNK)).astype(np.float32)
    kramp = np.ascontiguousarray(np.broadcast_to(ramp, (P, S)))
    in_maps = []
    for c in range(ncores):
        rs = slice(c * rows_per_core, (c + 1) * rows_per_core)
        in_maps.append(
            {
                "pred": pred[rs].reshape(P, Q * 16),
                "tgt": target[rs].reshape(P, Q),
                "wgt": weight[rs].reshape(P, Q // 4),
                "ident": ident,
                "kramp": kramp,
            }
        )
    return in_maps


def combine_partials(results, rows, host_dtype=np.float64):
    tot = host_dtype(0.0)
    for r in results:
        o = np.asarray(r["out"], host_dtype)
        tot = tot + (o[:, 0] - o[:, 1]).sum(dtype=host_dtype)
    return np.float32(tot / (rows * 4))


def kernel(pred, target, weight):
    global _BUILT
    from concourse.bass_utils import run_bass_kernel_spmd

    if _BUILT is None:
        _BUILT = build_nc()
    nc = _BUILT
    in_maps = make_host_inputs(pred, target, weight)
    res = run_bass_kernel_spmd(nc, in_maps, list(range(NCORES)))
    return combine_partials(res.results, pred.shape[0])


# revision 22
# speedup vs baseline: 1.9111x; 1.9111x over previous
"""DFL (distribution focal loss) Trainium2 Bass kernel.

Problem: pred [N,64] f32 (N=1048576, 4 sides x 16 bins), target [N,4] f32 in
[0,16), weight [N,1] f32.  loss = mean over [N,4] of w * KL(twohot(t) || softmax(p)).

Per-site (site = one (row, side), 16 bins):
    f   = frac(t);  l = floor(t)
    ent = f*log f + (1-f)*log(1-f)          (xlogy semantics)
    lse = log(sum_k exp(p_k))
    dot = sum_k p_k * relu(1 - |t - k|)     (= (1-f) p_l + f p_{l+1})
    kl  = ent + lse - dot
loss = sum(w*kl) / (4N)

Sharding: rows split across 8 cores (trivially data-parallel); scalar partials
are summed on host.

Device layout (per core): sites = rows*4 = 524288, viewed as [128 partitions,
Q=4096 sites], each partition owning contiguous sites. Per tile of S sites per
partition:
  - ACT: e = exp(p) (bf16 out), lse = log(s), entropy logs
  - DVE: one fused custom op  y = p * relu(min(u2, 2-u2)), u2 = t' - Idx,
    where t' = t + 1 + 16*(site % CHUNK) streams via a stride-0 broadcast AP
  - PE : segmented 16->1 sums of e and y as 16 PSUM-accumulating identity
    matmuls over strided views (out[m,n] = rhs[m,n] with lhsT = I)
  - DVE tensor_tensor_reduce with chained per-partition accumulators folds
    w * (ent + lse) and w * dot into [128,1] partials; host sums them.
"""

import numpy as np

REG_MAX = 16
N_ROWS = 1_048_576
NCORES = 8
P = 128
ROWS_PER_CORE = N_ROWS // NCORES          # 131072
Q_FULL = ROWS_PER_CORE * 4 // P           # 4096 sites per partition
S_FULL = 512                              # sites per partition per tile
CHUNK_FULL = 128                          # sites per custom-op instruction
EPS = 1e-12

_DFL_OP = None
_FRAC_OP = None
_BUILT = None


def _register_op(name, spec):
    """Register a custom DVE op at runtime, auto-pinning its uops sha."""
    import concourse.dve_ops as dve_ops

    for o in dve_ops.OPS:
        if o.name == name:
            return o
    dve_ops._SUB_OPCODE_FOR_NAME[name] = (
        max(dve_ops._SUB_OPCODE_FOR_NAME.values()) + 1
    )
    assert dve_ops._SUB_OPCODE_FOR_NAME[name] < 0x20
    shas = {}
    for ver in ("v3", "v4"):
        probe = dve_ops.DveOp(name, spec, subdim=False, uops_sha={})
        try:
            probe.compile(ver)
        except ValueError as e:
            import re

            m = re.search(r"\(%s: ([0-9a-f]+)" % ver, str(e))
            if m is None:
                raise
            shas[ver] = m.group(1)
        dve_ops._COMPILE_CACHE.pop((name, ver), None)
    op = dve_ops.DveOp(name, spec, subdim=False, uops_sha=shas)
    dve_ops.OPS.append(op)
    dve_ops.CUSTOM_DVE_SPECS[name] = spec
    return op


def _get_dfl_op():
    """Register the fused two-hot dot op: out = Src0 * relu(min(u2, 2-u2)),
    u2 = (Src1 + C0) - Idx. With Src1 = 1 + t + 2*site and C0 = -2j (pair j),
    element (s, b) [Idx = 2s+b] yields p * relu(1 - |t - k|) for k = 2j+b."""
    global _DFL_OP
    if _DFL_OP is not None:
        return _DFL_OP
    from concourse.dve_spec import Spec, Src0, Src1, Zero, One, Idx, C0, maxx, minn

    u2 = (Src1 + C0) - Idx
    two = One + One
    hat = maxx(minn(u2, two - u2), Zero)

    def _ref(in0, in1, c0, c1, c2):
        x = np.asarray(in0, np.float32)
        pp = x.shape[0]
        xf = x.reshape(pp, -1)
        tb = np.asarray(in1, np.float32).reshape(pp, -1)
        c = np.float32(c0 if not isinstance(c0, np.ndarray) else c0.flat[0])
        idx = np.arange(xf.shape[1], dtype=np.float32)[None, :]
        u2v = (tb + c) - idx
        hatv = np.maximum(np.minimum(u2v, 2.0 - u2v), 0.0)
        return (xf.astype(np.float32) * hatv).reshape(x.shape)

    _DFL_OP = _register_op("DFL_TWOHOT2_ANT", Spec(body=Src0 * hat, reference=_ref))
    return _DFL_OP


def _get_frac_op():
    """frac(x) for x in [0, 2^22): r = (x + 2^23) - 2^23 rounds to nearest
    integer; out = (x - r) + (r > x) = x - floor(x)."""
    global _FRAC_OP
    if _FRAC_OP is not None:
        return _FRAC_OP
    from concourse.dve_spec import Spec, Src0, C0

    r = (Src0 + C0) - C0
    body = (Src0 - r) + (r > Src0)

    def _ref(in0, in1, c0, c1, c2):
        x = np.asarray(in0, np.float32)
        c = np.float32(c0 if not isinstance(c0, np.ndarray) else c0.flat[0])
        rr = ((x + c).astype(np.float32) - c).astype(np.float32)
        return ((x - rr) + (rr > x)).astype(np.float32)

    _FRAC_OP = _register_op("FRAC_FLOOR_ANT", Spec(body=body, reference=_ref))
    return _FRAC_OP


def build_nc(Q=Q_FULL, S=S_FULL, CHUNK=CHUNK_FULL):
    """Build the single-core Bass program (same NEFF on all cores)."""
    import concourse.bass as bass
    import concourse.bacc as bacc
    import concourse.tile as tile
    from concourse import mybir

    f32 = mybir.dt.float32
    bf16 = mybir.dt.bfloat16
    AF = mybir.ActivationFunctionType
    ALU = mybir.AluOpType
    T = Q // S
    R = S // 4
    F16 = S * 16
    assert Q % S == 0 and S % CHUNK == 0 and S % 4 == 0 and CHUNK % 4 == 0

    from concourse.dve_ops import TENSOR_TENSOR_REDUCE as ttr_op

    # Steer ACT table-set choice to the one set containing BOTH exp and ln,
    # so Exp/Ln interleaving doesn't thrash ACT_TABLE_LOAD (~2.7us each).
    # Indices (= act_func_set_id) are preserved; single-function sets are
    # emptied so the chooser can't pick them.
    import concourse.hw_specs as hw_specs

    if not getattr(bacc, "_dfl_act_tables_patched", False):
        _orig_gat = hw_specs.get_activation_tables

        def _gat(arch):
            tabs = _orig_gat(arch)
            AF_ = mybir.ActivationFunctionType
            combined = {
                n for n, fns in tabs.items() if AF_.Exp in fns and AF_.Ln in fns
            }
            if combined:
                out = {}
                for n, fns in tabs.items():
                    if (AF_.Exp in fns or AF_.Ln in fns) and n not in combined:
                        fns = set()
                    out[n] = fns
                return out
            return tabs

        bacc.get_activation_tables = _gat
        bacc._dfl_act_tables_patched = True

    op = _get_dfl_op()
    frac_op = _get_frac_op()
    nc = bacc.Bacc("TRN2")
    pred_d = nc.dram_tensor("pred", [P, Q * 16], f32, kind="ExternalInput")
    t_d = nc.dram_tensor("tgt", [P, Q], f32, kind="ExternalInput")
    w_d = nc.dram_tensor("wgt", [P, Q // 4], f32, kind="ExternalInput")
    id_d = nc.dram_tensor("ident", [P, P], bf16, kind="ExternalInput")
    kr_d = nc.dram_tensor("kramp", [P, S], f32, kind="ExternalInput")
    out_d = nc.dram_tensor("out", [P, 2], f32, kind="ExternalOutput")

    with tile.TileContext(nc) as tc:
        with (
            tc.tile_pool(name="big", bufs=2) as big,
            tc.tile_pool(name="med", bufs=2) as med,
            tc.tile_pool(name="small", bufs=2) as small,
            tc.tile_pool(name="consts", bufs=1) as consts,
            tc.tile_pool(name="accp", bufs=1) as accp,
            tc.tile_pool(name="psum", bufs=2, space="PSUM") as psum,
        ):
            ident = consts.tile([P, P], bf16, tag="ident")
            nc.gpsimd.dma_start(ident[:], id_d[:])
            kramp = consts.tile([P, S], f32, tag="kramp")
            nc.gpsimd.dma_start(kramp[:], kr_d[:])
            eps_b = consts.tile([P, 1], f32, tag="eps_b")
            nc.gpsimd.memset(eps_b[:], EPS)
            one_b = consts.tile([P, 1], f32, tag="one_b")
            nc.gpsimd.memset(one_b[:], 1.0 + EPS)
            accA = accp.tile([P, T], f32, tag="accA")
            accB = accp.tile([P, 2 * T], f32, tag="accB")
            junk = accp.tile([P, S], f32, tag="junk")

            for t in range(T):
                pt = big.tile([P, F16], f32, tag="pred")
                nc.gpsimd.dma_start(pt[:], pred_d[:, t * F16 : (t + 1) * F16])
                tt = small.tile([P, S], f32, tag="t")
                nc.gpsimd.dma_start(tt[:], t_d[:, t * S : (t + 1) * S])
                wt = small.tile([P, R], f32, tag="w")
                nc.gpsimd.dma_start(wt[:], w_d[:, t * R : (t + 1) * R])

                # Pair-major layout: address(s, k) = (k//2)*2S + 2s + (k&1).
                # 4-byte-granule strided writes (~6% ACT penalty); every
                # k-pair group j is a contiguous [P, 2S] block, so PE matmuls
                # and DVE tree adds stream contiguously.
                e = med.tile([P, F16], bf16, tag="e")
                e_pm = e[:].rearrange("p (h s two) -> p s h two", h=8, two=2)
                nc.scalar.activation(e_pm, pt[:], AF.Exp)

                # tp2 = t + 1 + 2*site  (integer shift: frac(tp2) == frac(t))
                tp = small.tile([P, S], f32, tag="tp")
                nc.gpsimd.tensor_add(tp[:], tt[:], kramp[:])

                # y[s, k] = p[s, k] * relu(1 - |t - k|), written pair-major
                y = med.tile([P, F16], bf16, tag="y")
                pv = pt[:].rearrange("p (s k) -> p s k", k=16)
                yv = y[:].rearrange("p (h s two) -> p h s two", h=8, two=2)
                in1 = tp[:].unsqueeze(2).broadcast_to([P, S, 2])
                for j in range(8):
                    nc.vector._custom_dve(
                        op,
                        out=yv[:, j, :, :],
                        in0=pv[:, :, 2 * j : 2 * j + 2],
                        in1=in1,
                        s0=float(-2 * j),
                    )

                # dot-sums on PE: accumulate the 8 pair-groups; psum half h2
                # holds columns (2*s_local + b) for 256 sites.
                S2 = S * 2
                for h2 in range(2):
                    ydp = psum.tile([P, S], f32, tag=f"yd{h2}")
                    for j in range(8):
                        base = j * S2 + h2 * S
                        nc.tensor.matmul(
                            ydp[:], ident[:], y[:, base : base + S],
                            start=(j == 0), stop=(j == 7),
                        )
                    # T3 partial: sum_s w_s * (z[2s] + z[2s+1]) with w per row
                    # of 4 sites -> broadcast over 8 consecutive columns.
                    R2 = S // 8  # rows in this half (64)
                    wslice = wt[:, h2 * R2 : (h2 + 1) * R2]
                    wb8 = wslice.unsqueeze(2).broadcast_to([P, R2, 8])
                    zv = ydp[:].rearrange("p (r c) -> p r c", c=8)
                    jv8 = junk[:, 0 : S].rearrange("p (r c) -> p r c", c=8)
                    nc.vector._custom_dve(
                        ttr_op, out=jv8, in0=zv, in1=wb8, s0=0.0, s1=1.0,
                        accum_out=accB[:, 2 * t + h2 : 2 * t + h2 + 1],
                    )

                # exp-sums: bf16 tree, all-contiguous stages in pair-major
                z1 = med.tile([P, F16 // 2], bf16, tag="z1")
                nc.vector.tensor_add(z1[:], e[:, 0 : 8 * S], e[:, 8 * S : 16 * S])
                z2 = med.tile([P, F16 // 4], bf16, tag="z2")
                nc.vector.tensor_add(z2[:], z1[:, 0 : 4 * S], z1[:, 4 * S : 8 * S])
                z3 = med.tile([P, F16 // 8], bf16, tag="z3")
                nc.vector.tensor_add(z3[:], z2[:, 0 : 2 * S], z2[:, 2 * S : 4 * S])
                sums = small.tile([P, S], f32, tag="sums")
                z3v = z3[:].rearrange("p (s two) -> p s two", two=2)
                nc.vector.tensor_add(sums[:], z3v[:, :, 0], z3v[:, :, 1])

                lse = small.tile([P, S], f32, tag="lse")
                nc.scalar.activation(lse[:], sums[:], AF.Ln)

                fr = small.tile([P, S], f32, tag="fr")
                nc.vector._custom_dve(
                    frac_op, out=fr[:], in0=tp[:], s0=float(2**23)
                )
                lf = small.tile([P, S], f32, tag="lf")
                nc.scalar.activation(lf[:], fr[:], AF.Ln, bias=eps_b[:])
                l1f = small.tile([P, S], f32, tag="l1f")
                nc.scalar.activation(l1f[:], fr[:], AF.Ln, bias=one_b[:], scale=-1.0)

                d2 = small.tile([P, S], f32, tag="d2")
                nc.gpsimd.tensor_sub(d2[:], lf[:], l1f[:])
                m1 = small.tile([P, S], f32, tag="m1")
                nc.gpsimd.tensor_mul(m1[:], fr[:], d2[:])
                ent = small.tile([P, S], f32, tag="ent")
                nc.gpsimd.tensor_add(ent[:], m1[:], l1f[:])
                kl = small.tile([P, S], f32, tag="kl")
                nc.gpsimd.tensor_add(kl[:], ent[:], lse[:])

                wb = wt[:].unsqueeze(2).broadcast_to([P, R, 4])
                klv = kl[:].rearrange("p (r j) -> p r j", j=4)
                jv = junk[:].rearrange("p (r j) -> p r j", j=4)
                nc.vector._custom_dve(
                    ttr_op, out=jv, in0=klv, in1=wb, s0=0.0, s1=1.0,
                    accum_out=accA[:, t : t + 1],
                )

            sumA = accp.tile([P, 1], f32, tag="sumA")
            sumB = accp.tile([P, 1], f32, tag="sumB")
            nc.vector.tensor_reduce(
                sumA[:], accA[:], axis=mybir.AxisListType.X, op=ALU.add
            )
            nc.vector.tensor_reduce(
                sumB[:], accB[:], axis=mybir.AxisListType.X, op=ALU.add
            )
            outt = accp.tile([P, 2], f32, tag="outt")
            nc.vector.tensor_copy(outt[:, 0:1], sumA[:])
            nc.vector.tensor_copy(outt[:, 1:2], sumB[:])
            nc.gpsimd.dma_start(out_d[:], outt[:])

    nc.finalize()
    return nc


def make_host_inputs(pred, target, weight, Q=Q_FULL, S=S_FULL, CHUNK=CHUNK_FULL,
                     ncores=NCORES):
    """Shard full inputs into per-core in_maps."""
    import ml_dtypes

    pred = np.ascontiguousarray(np.asarray(pred, np.float32))
    target = np.ascontiguousarray(np.asarray(target, np.float32))
    weight = np.ascontiguousarray(np.asarray(weight, np.float32))
    rows = pred.shape[0]
    rows_per_core = rows // ncores
    ident = np.eye(P, dtype=ml_dtypes.bfloat16)
    ramp = (1.0 + 2.0 * np.arange(S)).astype(np.float32)
    kramp = np.ascontiguousarray(np.broadcast_to(ramp, (P, S)))
    in_maps = []
    for c in range(ncores):
        rs = slice(c * rows_per_core, (c + 1) * rows_per_core)
        in_maps.append(
            {
                "pred": pred[rs].reshape(P, Q * 16),
                "tgt": target[rs].reshape(P, Q),
                "wgt": weight[rs].reshape(P, Q // 4),
                "ident": ident,
                "kramp": kramp,
            }
        )
    return in_maps


def combine_partials(results, rows, host_dtype=np.float64):
    tot = host_dtype(0.0)
    for r in results:
        o = np.asarray(r["out"], host_dtype)
        tot = tot + (o[:, 0] - o[:, 1]).sum(dtype=host_dtype)
    return np.float32(tot / (rows * 4))


def kernel(pred, target, weight):
    global _BUILT
    from concourse.bass_utils import run_bass_kernel_spmd

    if _BUILT is None:
        _BUILT = build_nc()
    nc = _BUILT
    in_maps = make_host_inputs(pred, target, weight)
    res = run_bass_kernel_spmd(nc, in_maps, list(range(NCORES)))
    return combine_partials(res.results, pred.shape[0])
